# revision 5
# baseline (speedup 1.0000x reference)
"""Trainium2 Bass kernel for nn_AMTCL_77867757077077 (AMTCL triplet-center loss).

Key insight: the reference's [B,B] distance matrix dist[i,j] depends on j only
through targets[j], so it has just C=100 distinct columns:

    dist[i,j] = S[i, t_j],  S[i,k] = sqrt(q[k] - 2*(x @ u.T)[i,k] + (x^2 @ v.T)[i,k])

with v = 2^w, u = v*c, q[k] = sum_d v[k,d]*c[k,d]^2.  Then
    dist_ap[i] = S[i, t_i]
    dist_an[i] = min_{k != t_i, k present} S[i,k]
    per_sample = ap + relu(cc - an),  cc = centers_dist[t_i]
so the O(B^2 D) problem collapses to two [B,C] matmuls (O(B C D)).

Sharding: data-parallel over batch rows, 8 cores x 512 rows.  centers /
centers_weights replicated.  Each core emits its partial sum/B; the host adds
the 8 scalars (the "all-reduce" of the hint, done on 8 floats host-side).

Layout trick for the per-row gathers: everything stays in the matmul's native
[class k, row i] layout.  ohT[k,i] = (t_i == k) is one DVE op (broadcast t
minus per-partition iota, is_equal).  Then
    ap+cc per row = ones^T @ (ohT * (S^T + cd[k]))      (TensorE column-sum)
    cc per row    = cd^T @ ohT                          (TensorE)
    an per row    = row-min of transposed (S^T + 1e9*ohT) tiles
and [1,512] rows are flipped into [128,4] columns with tiny K=1 matmuls.

Everything numeric runs on device: 2^w, u, q, the closest-center distance
matrix (100x100 via matmuls), both [C,512] matmul chains (bf16 inputs, f32
PSUM accumulation), sqrt, one-hot gathers, row mins and the final reduction.
Host only reshapes/casts inputs (x->bf16 shards, c/w transposed, targets as
f32, absent-class penalty from bincount).
"""

import math
import numpy as np

NUM_CORES = 8
B = 4096
D = 384
C = 100
BL = B // NUM_CORES  # 512 rows per core
P = 128
NT = BL // P         # 4 row tiles per core
KD = D // P          # 3 contraction chunks

_CACHE = {}


def _build_nc():
    import concourse.bass as bass
    import concourse.bacc as bacc
    import concourse.tile as tile
    from concourse import mybir
    from concourse.masks import make_identity
    from contextlib import ExitStack

    f32 = mybir.dt.float32
    bf16 = mybir.dt.bfloat16
    LN2 = float(math.log(2.0))

    nc = bacc.Bacc(
        "TRN2", target_bir_lowering=False, debug=False, num_devices=NUM_CORES
    )

    x_ext = nc.dram_tensor("x", [BL, D], bf16, kind="ExternalInput").ap()
    t_ext = nc.dram_tensor("t", [BL], f32, kind="ExternalInput").ap()
    ct_ext = nc.dram_tensor("ct", [D, C], f32, kind="ExternalInput").ap()
    wt_ext = nc.dram_tensor("wt", [D, C], f32, kind="ExternalInput").ap()
    pen_ext = nc.dram_tensor("pen", [C, 1], f32, kind="ExternalInput").ap()
    out_ext = nc.dram_tensor("out", [1, 1], f32, kind="ExternalOutput").ap()

    with tile.TileContext(nc) as tc, ExitStack() as ctx:
        singles = ctx.enter_context(tc.tile_pool(name="singles", bufs=1))
        ps_big = ctx.enter_context(tc.tile_pool(name="psbig", bufs=1, space="PSUM"))
        ps_tr = ctx.enter_context(tc.tile_pool(name="pstr", bufs=2, space="PSUM"))
        ps_misc = ctx.enter_context(tc.tile_pool(name="psmisc", bufs=1, space="PSUM"))
        ps_g = ctx.enter_context(tc.tile_pool(name="psg", bufs=3, space="PSUM"))

        # ---- constants
        ident = singles.tile([P, P], f32)
        make_identity(nc, ident)
        iota_col = singles.tile([P, 1], f32)   # value = partition index
        nc.gpsimd.iota(
            iota_col,
            pattern=[[0, 1]],
            base=0,
            channel_multiplier=1,
            allow_small_or_imprecise_dtypes=True,
        )
        ones_col = singles.tile([P, 1], f32)
        nc.vector.memset(ones_col, 1.0)
        invB_col = singles.tile([P, 1], f32)
        nc.vector.memset(invB_col, 1.0 / B)
        one11 = singles.tile([1, 1], f32)
        nc.vector.memset(one11, 1.0)

        # ---- input DMAs
        xT = singles.tile([P, KD, BL], bf16)    # x^T, d-major (xbar transpose)
        for k in range(KD):
            nc.sync.dma_start_transpose(xT[:, k, :], x_ext[:, k * P : (k + 1) * P])
        ctT = singles.tile([P, KD, C], f32)     # c^T
        nc.sync.dma_start(ctT, ct_ext.rearrange("(k p) c -> p k c", p=P))
        wtT = singles.tile([P, KD, C], f32)     # w^T
        nc.sync.dma_start(wtT, wt_ext.rearrange("(k p) c -> p k c", p=P))
        # t broadcast to all partitions straight from DRAM (partition-step 0)
        tbc_sb = singles.tile([P, BL], f32)
        t_bcast_ap = bass.AP(
            tensor=t_ext.tensor,
            offset=t_ext.offset,
            ap=[[0, P], [1, BL]],
        )
        nc.sync.dma_start(out=tbc_sb, in_=t_bcast_ap)
        pen_sb = singles.tile([C, 1], f32)      # absent-class d2 penalty
        nc.sync.dma_start(pen_sb, pen_ext)

        # ---- center-side prep (all in transposed [d, class] layout)
        vT = singles.tile([P, KD, C], bf16)     # 2^w  = exp(ln2 * w)
        nc.scalar.activation(vT, wtT, mybir.ActivationFunctionType.Exp, scale=LN2)
        ct_bf = singles.tile([P, KD, C], bf16)
        nc.vector.tensor_copy(ct_bf, ctT)
        c2T = singles.tile([P, KD, C], bf16)    # c^2
        nc.vector.tensor_mul(c2T, ct_bf, ct_bf)
        uT2 = singles.tile([P, KD, C], bf16)    # -2 * v * c
        nc.vector.scalar_tensor_tensor(
            out=uT2,
            in0=vT,
            scalar=-2.0,
            in1=ct_bf,
            op0=mybir.AluOpType.mult,
            op1=mybir.AluOpType.mult,
        )
        qmat = singles.tile([P, KD, C], f32)    # v * c^2 (summed over d below)
        nc.vector.tensor_mul(qmat, vT, c2T)

        # q[k] = sum_d v[k,d] c[k,d]^2 as a [1,C] row via ones-matmul
        q_ps = ps_misc.tile([1, C], f32, tag="misc")
        for k in range(KD):
            nc.tensor.matmul(
                q_ps,
                lhsT=ones_col,
                rhs=qmat[:, k, :],
                start=(k == 0),
                stop=(k == KD - 1),
            )
        q_row = singles.tile([1, C], f32)
        nc.vector.tensor_copy(q_row, q_ps)
        # flip to a [C,1] per-partition column: q_col = q_row.T @ [[1]]
        qflip_ps = ps_misc.tile([C, 1], f32, tag="misc")
        nc.tensor.matmul(qflip_ps, lhsT=q_row, rhs=one11)
        q_col = singles.tile([C, 1], f32)
        nc.vector.tensor_copy(q_col, qflip_ps)
        qpen_col = singles.tile([C, 1], f32)
        nc.vector.tensor_add(qpen_col, q_col, pen_sb)

        # ---- closest-center distance cd[k] (100x100 matmul path)
        cd_ps = ps_big.tile([C, C], f32)
        for k in range(KD):
            nc.tensor.matmul(
                cd_ps, lhsT=vT[:, k, :], rhs=c2T[:, k, :],
                start=(k == 0), stop=False,
            )
        for k in range(KD):
            nc.tensor.matmul(
                cd_ps, lhsT=uT2[:, k, :], rhs=ct_bf[:, k, :],
                start=False, stop=(k == KD - 1),
            )
        # d2c = max(cd_ps + q[i], 0) then sqrt; diagonal -> 1e9; row-min
        e_sb = singles.tile([C, C], f32)
        nc.vector.tensor_scalar(
            out=e_sb, in0=cd_ps, scalar1=q_col[:, :], scalar2=0.0,
            op0=mybir.AluOpType.add, op1=mybir.AluOpType.max,
        )
        dd_sb = singles.tile([C, C], f32)
        nc.scalar.activation(dd_sb, e_sb, mybir.ActivationFunctionType.Sqrt)
        nc.gpsimd.affine_select(
            out=dd_sb, in_=dd_sb,
            compare_op=mybir.AluOpType.not_equal,
            fill=1e9, base=0, pattern=[[-1, C]], channel_multiplier=1,
        )
        cd_colP = singles.tile([P, 1], f32)
        nc.vector.memset(cd_colP, 0.0)
        nc.vector.tensor_reduce(
            cd_colP[:C, :], dd_sb, axis=mybir.AxisListType.X, op=mybir.AluOpType.min
        )

        # ---- x^2 (bf16, d-major)
        x2T = singles.tile([P, KD, BL], bf16)
        for k in range(KD):
            nc.vector.tensor_mul(x2T[:, k, :], xT[:, k, :], xT[:, k, :])

        # ---- main matmul chain: S^T[k_class, i] partial d2 in PSUM
        s_ps = ps_big.tile([C, BL], f32)
        for k in range(KD):
            nc.tensor.matmul(
                s_ps, lhsT=vT[:, k, :], rhs=x2T[:, k, :],
                start=(k == 0), stop=False,
            )
        for k in range(KD):
            nc.tensor.matmul(
                s_ps, lhsT=uT2[:, k, :], rhs=xT[:, k, :],
                start=False, stop=(k == KD - 1),
            )
        # S^T = sqrt(d2 + q + pen), rows 100..127 zeroed
        st_sb = singles.tile([P, BL], f32)
        nc.gpsimd.memset(st_sb, 0.0)
        nc.scalar.activation(
            st_sb[:C, :], s_ps, mybir.ActivationFunctionType.Sqrt, bias=qpen_col[:, :]
        )

        # ---- [k,i]-layout gathers
        # ohT[k,i] = ((t_bcast[k,i] - k) == 0)
        ohT = singles.tile([P, BL], f32)
        nc.vector.tensor_scalar(
            out=ohT, in0=tbc_sb, scalar1=iota_col[:, :], scalar2=0.0,
            op0=mybir.AluOpType.subtract, op1=mybir.AluOpType.is_equal,
        )
        # w1 = ohT * (S^T + cd[k]);  sbig = S^T + 1e9*ohT
        tmp_scd = singles.tile([P, BL], f32)
        nc.vector.tensor_scalar_add(tmp_scd, st_sb, cd_colP[:, :])
        w1 = singles.tile([P, BL], f32)
        nc.vector.tensor_mul(w1, tmp_scd, ohT)
        sbig = singles.tile([P, BL], f32)
        nc.vector.scalar_tensor_tensor(
            out=sbig, in0=ohT, scalar=1e9, in1=st_sb,
            op0=mybir.AluOpType.mult, op1=mybir.AluOpType.add,
        )
        # apcc_row[i] = S[i,t_i] + cd[t_i];  cc_row[i] = cd[t_i]
        apcc_ps = ps_g.tile([1, BL], f32, tag="g")
        nc.tensor.matmul(apcc_ps, lhsT=ones_col, rhs=w1)
        apcc_row = singles.tile([1, BL], f32)
        nc.scalar.copy(apcc_row, apcc_ps)
        cc_ps = ps_g.tile([1, BL], f32, tag="g")
        nc.tensor.matmul(cc_ps, lhsT=cd_colP, rhs=ohT)
        cc_row = singles.tile([1, BL], f32)
        nc.scalar.copy(cc_row, cc_ps)

        # columnize rows into [128, NT] (tiny K=1 matmuls), apcc | cc
        colcat = ps_g.tile([P, 2 * NT], f32, tag="g")
        for t in range(NT):
            nc.tensor.matmul(
                colcat[:, t : t + 1],
                lhsT=apcc_row[:, t * P : (t + 1) * P],
                rhs=one11,
            )
            nc.tensor.matmul(
                colcat[:, NT + t : NT + t + 1],
                lhsT=cc_row[:, t * P : (t + 1) * P],
                rhs=one11,
            )

        # an per row: transpose sbig tiles, row-min
        mnc = singles.tile([P, NT], f32)
        for t in range(NT):
            st_ps = ps_tr.tile([P, P], f32)
            nc.tensor.transpose(st_ps, sbig[:, t * P : (t + 1) * P], ident)
            nc.vector.tensor_reduce(
                mnc[:, t : t + 1], st_ps[:, :C], axis=mybir.AxisListType.X,
                op=mybir.AluOpType.min,
            )

        # per_sample = apcc - min(an, cc); sum / B
        rc = singles.tile([P, NT], f32)
        nc.vector.tensor_tensor(
            out=rc, in0=mnc, in1=colcat[:, NT : 2 * NT], op=mybir.AluOpType.min
        )
        junkc = singles.tile([P, NT], f32)
        total_col = singles.tile([P, 1], f32)
        nc.vector.scalar_tensor_tensor(
            out=junkc, in0=colcat[:, 0:NT], scalar=1.0, in1=rc,
            op0=mybir.AluOpType.mult, op1=mybir.AluOpType.subtract,
            accum_out=total_col,
        )
        # partition-sum via ones-matmul -> [1,1]; 1/B folded into the lhsT
        fin_ps = ps_misc.tile([1, 1], f32, tag="misc")
        nc.tensor.matmul(fin_ps, lhsT=invB_col, rhs=total_col)
        out_sb = singles.tile([1, 1], f32)
        nc.vector.tensor_copy(out_sb, fin_ps)
        nc.sync.dma_start(out_ext, out_sb)

    nc.compile()
    return nc


def _get_nc():
    if "nc" not in _CACHE:
        _CACHE["nc"] = _build_nc()
    return _CACHE["nc"]


def make_in_maps(inputs, targets, centers, centers_weights):
    import ml_dtypes

    x = np.asarray(inputs, np.float32)
    t = np.asarray(targets).astype(np.int64)
    c = np.asarray(centers, np.float32)
    w = np.asarray(centers_weights, np.float32)
    assert x.shape == (B, D) and c.shape == (C, D) and w.shape == (C, D)

    x_bf = x.astype(ml_dtypes.bfloat16)
    ct = np.ascontiguousarray(c.T)                      # [D, C] f32
    wt = np.ascontiguousarray(w.T)                      # [D, C] f32
    t_f32 = t.astype(np.float32)
    present = np.bincount(t, minlength=C) > 0
    pen = np.where(present, 0.0, 1e12).astype(np.float32).reshape(C, 1)

    in_maps = []
    for i in range(NUM_CORES):
        sl = slice(i * BL, (i + 1) * BL)
        in_maps.append(
            {
                "x": np.ascontiguousarray(x_bf[sl]),
                "t": np.ascontiguousarray(t_f32[sl]),
                "ct": ct,
                "wt": wt,
                "pen": pen,
            }
        )
    return in_maps


def run(inputs, targets, centers, centers_weights, trace=False):
    """Build+run the SPMD kernel; returns (loss_scalar, BassKernelResults)."""
    from concourse import bass_utils

    nc = _get_nc()
    in_maps = make_in_maps(inputs, targets, centers, centers_weights)
    res = bass_utils.run_bass_kernel_spmd(
        nc, in_maps, core_ids=list(range(NUM_CORES)), trace=trace
    )
    loss = np.float32(0.0)
    for r in res.results:
        loss += np.float32(r["out"][0, 0])
    return np.array(loss, dtype=np.float32), res


def kernel(inputs, targets, epoch_number=None, centers=None, centers_weights=None):
    loss, _ = run(inputs, targets, centers, centers_weights, trace=False)
    return loss
